# revision 27
# baseline (speedup 1.0000x reference)
"""MoE (brute-force reference) kernel for 8 TRN2 NeuronCores.

Strategy: expert-parallel. Host routes token-slots by gate_idx to their
expert, dedups tokens whose two top-k picks agree, and transposes so the
device sees xt[s] = X_e.T [D, Cs]. Each core owns 2 expert slots and computes
  hT[m] = gelu(sum_k w1T[k,m].T @ xT[k] + b1)   then
  yT[m] = sum_k w2T[k,m].T @ hT[k]
All matmul operands are fp16 (full PE rate, ~8x fp8 accuracy);
accumulation is fp32 in PSUM. Weights stream from HBM as fp16 (halves
DMA vs fp32). b1 is applied on-device (per-partition bias fused into
the gelu activation); b2 and the gate_score combine happen on host in
exact fp32.

Slot capacities are derived from the actual routing at runtime: experts
are sorted by token count and paired largest-with-smallest onto cores,
so slot 0 holds the biggest expert of each core (capacity C0 = global
max count) and slot 1 the smaller one (C1 = max count of the small
half). The Bass program is compiled per (C0, C1) and cached. No PE
warm-up: the HAM clock ramps during the first real matmuls; dummy
matmuls only delay the stream.
"""

import numpy as np

import concourse.bacc as bacc
import concourse.mybir as mybir
from concourse import tile
from concourse.bass_utils import run_bass_kernel_spmd

E, D, H, TOPK, T = 16, 1024, 2048, 2, 2048
NCORES = 8
EPC = E // NCORES  # expert slots per core
KD, KH, MD = D // 128, H // 128, D // 128  # 8, 16, 8

_F16 = np.float16
_CACHE: dict = {}
_RUN_KW: dict = {}   # extra kwargs for run_bass_kernel_spmd (test-only hook)
_LAST = [None]       # last BassKernelResults (test-only hook)


def _build(caps):
    """caps: per-slot token capacities, len EPC. Compiled/cached per caps."""
    CM = max(caps)
    dt = mybir.dt.float16
    f32 = mybir.dt.float32
    nc = bacc.Bacc("TRN2", target_bir_lowering=False, debug=False,
                   num_devices=NCORES)
    xt = nc.dram_tensor("xt", [EPC, D, CM], dt, kind="ExternalInput")
    w1t = nc.dram_tensor("w1t", [EPC, D, H], dt, kind="ExternalInput")
    w2t = nc.dram_tensor("w2t", [EPC, H, D], dt, kind="ExternalInput")
    b1 = nc.dram_tensor("b1", [EPC, 128, KH], f32, kind="ExternalInput")
    yt = nc.dram_tensor("yt", [EPC, D, CM], dt, kind="ExternalOutput")

    gelu = mybir.ActivationFunctionType.Gelu_apprx_tanh
    MGRP = 8   # GEMM1 m-tiles per psum group (k-inner within a group)

    with tile.TileContext(nc) as tc:
        with (
            tc.tile_pool(name="xtp", bufs=2) as xtp,
            tc.tile_pool(name="w1p", bufs=2) as w1p,
            tc.tile_pool(name="w2p", bufs=2) as w2p,
            tc.tile_pool(name="htp", bufs=2) as htp,
            tc.tile_pool(name="yp", bufs=16) as yp,
            tc.tile_pool(name="bp", bufs=2) as bp,
            tc.tile_pool(name="ps", bufs=1, space="PSUM") as psp,
        ):
            for e in range(EPC):
                C = caps[e]
                u = f"{e}"
                # Both HWDGE queues (sync + scalar) stream inputs and
                # weights in need order, round-robin so neither queue's
                # FIFO falls behind consumption. gpsimd (SWDGE, slow)
                # only carries mid-kernel y evictions.
                xin = xt.ap()[e].rearrange("(k p) c -> p k c", p=128)
                hk = KD // 2
                xth = [xtp.tile([128, hk * CM], dt, name=f"xt{u}_{i}",
                                tag=f"xt{i}") for i in range(2)]

                def xt_dma(eng, half, k0, nk):
                    base = half * hk
                    eng.dma_start(
                        out=xth[half][:].rearrange(
                            "p (k c) -> p k c", c=CM)[:, k0:k0 + nk, :C],
                        in_=xin[:, base + k0:base + k0 + nk, :C])

                def xtv(k):
                    return xth[k // hk][:, (k % hk) * CM:(k % hk) * CM + C]

                # weight piece loader: piece = (eng, k0, nk, m0, nm, tag)
                # covering k-tiles [k0,k0+nk) x m-tiles [m0,m0+nm).
                # Returns {(k, m): AP [128, 128]}.
                def wpieces(pool, pfx, dram, pieces):
                    views = {}
                    for (eng, k0, nk, m0, nm, tag) in pieces:
                        tl = pool.tile([128, nk, nm * 128], dt,
                                       name=f"{pfx}{u}_{tag}",
                                       tag=f"{pfx}{tag}")
                        eng.dma_start(
                            out=tl[:],
                            in_=dram.ap()[e, k0 * 128:(k0 + nk) * 128,
                                          m0 * 128:(m0 + nm) * 128]
                                .rearrange("(k p) m -> p k m", p=128))
                        for k in range(k0, k0 + nk):
                            for m in range(m0, m0 + nm):
                                views[(k, m)] = tl[:, k - k0,
                                                   (m - m0) * 128:
                                                   (m - m0 + 1) * 128]
                    return views

                sy, sc = nc.sync, nc.scalar
                # Scalar (Act) engine runs the gelus, so its HWDGE issue
                # count must stay under the ~8 in-flight semaphore limit
                # before each gelu batch or the issue's reuse-wait blocks
                # the FIFO and starves GEMM2. Scalar: xt + b1 + 3 light
                # w1 slabs up front, 2 late w2 chunks between gelu
                # batches. Sync: everything else, in need order (pure DMA
                # FIFO, blocking just paces the stream).
                if e == 0:
                    # fast start: first slab split in half so matmul 0
                    # fires as soon as xt k0-1 (scalar q) + w1a m0-3
                    # (sync q) land, in parallel on the two queues.
                    # GEMM1 group 0 is m0-3 only, so the second half
                    # (ab) has slack to arrive.
                    w1_pieces = [(sy, 0, 1, 0, 4, "aa"), (sy, 0, 1, 4, 4, "ab")]
                    xt_dma(sc, 0, 0, 2)
                    xt_dma(sc, 0, 2, 2)
                    w1v_a1 = wpieces(w1p, "w1", w1t, [(sc, 1, 1, 0, 8, "a1")])
                else:
                    w1_pieces = [(sy, 0, 1, 0, 8, "a0"),
                                 (sy, 1, 1, 0, 8, "a1")]
                    w1v_a1 = {}
                    xt_dma(sc, 0, 0, 4)
                xt_dma(sc, 1, 0, 4)
                b1s = bp.tile([128, KH], f32, name=f"b1s{u}", tag="b1s")
                sc.dma_start(out=b1s[:], in_=b1.ap()[e])
                ae = sc if e == 0 else sy  # mid w1a slabs: spread on e0
                w1_pieces += [
                    (sy, 2, 1, 0, 8, "a2"),
                    (sy, 3, 1, 0, 8, "a3"), (sy, 4, 1, 0, 8, "a4"),
                    (ae, 5, 1, 0, 8, "a5"), (ae, 6, 1, 0, 8, "a6"),
                    (ae, 7, 1, 0, 8, "a7"),
                    (sy, 0, 2, 8, 8, "b0"), (sy, 2, 2, 8, 8, "b1"),
                    (sy, 4, 2, 8, 8, "b2"), (sc, 6, 2, 8, 8, "b3"),
                ]
                w1v = wpieces(w1p, "w1", w1t, w1_pieces)
                w1v.update(w1v_a1)
                w2v = wpieces(w2p, "w2", w2t, [
                    (sy, 0, 2, 0, 8, "c0"), (sy, 2, 2, 0, 8, "c1"),
                    (sy, 4, 2, 0, 8, "c2"), (sy, 8, 2, 0, 8, "c4"),
                    (sy, 12, 2, 0, 8, "c6"),
                ])

                # GEMM1: hT[m] = gelu(sum_k w1[k,m].T @ xts[k] + b1)
                hts = [htp.tile([128, CM], dt, name=f"ht{u}_{m}",
                                tag=f"ht{m}") for m in range(KH)]
                groups = ([(0, 4), (4, 4), (8, 8)] if e == 0
                          else [(0, 8), (8, 8)])
                for g, gw in groups:
                    pss = [psp.tile([128, CM], f32, name=f"ps1_{u}_{m}",
                                    tag=f"ps{m % MGRP}")
                           for m in range(g, g + gw)]
                    for k in range(KD):
                        for i, m in enumerate(range(g, g + gw)):
                            nc.tensor.matmul(
                                pss[i][:, :C], w1v[(k, m)], xtv(k),
                                start=(k == 0), stop=(k == KD - 1))
                    for i, m in enumerate(range(g, g + gw)):
                        nc.scalar.activation(
                            hts[m][:, :C], pss[i][:, :C], gelu,
                            bias=b1s[:, m:m + 1])
                    if g + gw == MGRP:
                        # late w2 chunks ride the scalar queue between
                        # gelu batches (issue slots are free again here)
                        w2v.update(wpieces(w2p, "w2", w2t, [
                            (sc, 6, 2, 0, 8, "c3"),
                            (sc, 10, 2, 0, 8, "c5"),
                            (sc, 14, 2, 0, 8, "c7"),
                        ]))

                # GEMM2: yT[m] = sum_k w2[k,m].T @ hts[k]
                # Two k-phases (k0-7, then k8-15) so the weight need
                # order matches the round-robin arrival order; evictions
                # stream per-m during phase 2.
                ytv = yt.ap()[e].rearrange("(g p) c -> p g c", p=128)
                ps2 = [psp.tile([128, CM], f32, name=f"ps2_{u}_{m}",
                                tag=f"ps{m % MGRP}") for m in range(MD)]
                for k in range(KH // 2):
                    for m in range(MD):
                        nc.tensor.matmul(
                            ps2[m][:, :C], w2v[(k, m)], hts[k][:, :C],
                            start=(k == 0), stop=False)
                for m in range(MD):
                    for k in range(KH // 2, KH):
                        nc.tensor.matmul(
                            ps2[m][:, :C], w2v[(k, m)], hts[k][:, :C],
                            start=False, stop=(k == KH - 1))
                    yo = yp.tile([128, CM], dt, name=f"y{u}_{m}", tag="y")
                    nc.vector.tensor_copy(out=yo[:, :C], in_=ps2[m][:, :C])
                    # SWDGE keeps y off the weight chain mid-kernel, but
                    # its ~3us end-drain would sit on the critical tail:
                    # the last expert's outputs take the idle HWDGE.
                    y_eng = nc.sync if e == EPC - 1 else nc.gpsimd
                    y_eng.dma_start(
                        out=ytv[:, m, :C],
                        in_=yo[:, :C])
    nc.compile()
    return nc


def _get_nc(caps):
    key = tuple(caps)
    if key not in _CACHE:
        _CACHE[key] = _build(key)
    return _CACHE[key]


def _route(gate_idx, gate_score):
    """Dedup routing: tokens whose two top-k picks are the same expert are
    sent once with summed score. Returns per-expert (tokens, weights)."""
    g = np.asarray(gate_idx).astype(np.int64)
    sc = np.asarray(gate_score, dtype=np.float32)
    out = []
    for e in range(E):
        m0, m1 = g[:, 0] == e, g[:, 1] == e
        toks = np.flatnonzero(m0 | m1)
        wts = (sc[:, 0] * m0 + sc[:, 1] * m1)[toks]
        out.append((toks, wts))
    return out


def kernel(inp, gate_idx, gate_score, w1, b1, w2, b2):
    inp = np.asarray(inp, dtype=np.float32)
    gate_idx = np.asarray(gate_idx)
    gate_score = np.asarray(gate_score, dtype=np.float32)
    w1 = np.asarray(w1, dtype=np.float32)
    b1 = np.asarray(b1, dtype=np.float32)
    w2 = np.asarray(w2, dtype=np.float32)
    b2 = np.asarray(b2, dtype=np.float32)

    routes = _route(gate_idx, gate_score)
    counts = np.array([len(r[0]) for r in routes])

    # Largest-with-smallest pairing: slot s of core c gets expert
    # order[c] (s=0) / order[2*NCORES-1-c] (s=1). Slot capacity is the
    # max count over cores for that slot.
    order = np.argsort(-counts, kind="stable")
    expert_of = np.empty((NCORES, EPC), dtype=np.int64)
    for c in range(NCORES):
        expert_of[c, 0] = order[c]
        expert_of[c, 1] = order[2 * NCORES - 1 - c]
    caps = tuple(int(max(counts[expert_of[c, s]] for c in range(NCORES)))
                 for s in range(EPC))
    CM = max(caps)

    # Host-side gather + transpose, cast to fp16 for the device.
    w1t_all = np.ascontiguousarray(
        w1.transpose(0, 2, 1)).astype(_F16)  # [E, D, H]
    w2t_all = np.ascontiguousarray(
        w2.transpose(0, 2, 1)).astype(_F16)  # [E, H, D]
    b1r = np.ascontiguousarray(
        b1.reshape(E, KH, 128).transpose(0, 2, 1))  # [E, 128, KH]

    in_maps = []
    for c in range(NCORES):
        es = expert_of[c]
        xt_c = np.zeros((EPC, D, CM), dtype=_F16)
        for s, e in enumerate(es):
            toks = routes[e][0][:caps[s]]
            if len(toks):
                xt_c[s, :, :len(toks)] = inp[toks].T.astype(_F16)
        in_maps.append({
            "xt": xt_c,
            "w1t": np.ascontiguousarray(w1t_all[es]),
            "w2t": np.ascontiguousarray(w2t_all[es]),
            "b1": np.ascontiguousarray(b1r[es]),
        })

    nc = _get_nc(caps)
    res = run_bass_kernel_spmd(nc, in_maps, list(range(NCORES)), **_RUN_KW)
    _LAST[0] = res

    # Host combine: weight each expert's output columns by the (summed)
    # gate score and accumulate per token; add the b2 term (folded out of
    # the device kernel). Tokens are unique within an expert, so the
    # fancy-indexed += is safe.
    out = np.einsum("tk,tkd->td", gate_score,
                    b2[gate_idx.astype(np.int64)])
    out = np.ascontiguousarray(out, dtype=np.float32)
    for c in range(NCORES):
        for s in range(EPC):
            e = int(expert_of[c, s])
            toks, wts = routes[e]
            n = min(len(toks), caps[s])
            if n:
                y = res.results[c]["yt"][s, :, :n].T.astype(np.float32)
                out[toks[:n]] += wts[:n, None] * y
            if len(toks) > n:  # exact host fallback, never hit in practice
                ot, ow = toks[n:], wts[n:]
                hh = inp[ot] @ w1[e].T + b1[e]
                hh = 0.5 * hh * (1.0 + np.tanh(
                    np.sqrt(2.0 / np.pi) * (hh + 0.044715 * hh ** 3)))
                out[ot] += ow[:, None] * (hh @ w2[e].T)
    return out
